# revision 1
# baseline (speedup 1.0000x reference)
"""CascadeHadamardSmoothLinear Trainium2 kernel (v2).

out = Q_nvfp4(hadamard_rotate(x * smooth_scale * S_in)) @ W.T + bias

Sharding: data-parallel over batch*seq rows across 8 cores; everything else
replicated.  Host stages transposed layouts so the device never transposes
the big operands:
  - x is fed as xT [Din, rows]  (f32)   -> rotation lhsT tiles read directly
  - w is fed as wT [Din, Dout]  (f32)   -> main-matmul stationary read directly
  - output is produced as outT [Dout, rows] and transposed back on host.

Device pipeline per core (rows=512, Din=Dout=4096):
  - hp[k, b*128+l] = H[k,l] * smooth[k,b] * S_in[k,b]   (fp32, folded scales)
  - rotation: rps[m, l] = xT_b.T @ hp_b   (fp32 matmul, exact)
  - NVFP4 fake-quant of rps (exact snap arithmetic, spread over DVE/ACT/GPSIMD)
  - PE-transpose quantized tiles into xqt[b] = [k, m] bf16
  - main: accT[n, m] += wT_tile(natural [k,n] slice, stationary) @ xqt[b]
    accumulated over all 32 k-blocks in PSUM; bias added via ACT with
    per-partition bias vector; outT stored [Dout, rows].
The first output group's accumulation is interleaved with prep so the PE
stays busy while DVE quantizes; W tiles stream in bf16 via SWDGE cast DMA,
double buffered.
"""
from contextlib import ExitStack

import numpy as np

F32_M_MAGIC = 12582912.0  # 1.5 * 2**23: add+sub rounds f32 to int (RNE)

_CACHE = {}


def _build(rows, din, dout, repeat=1):
    """Build the per-core Bass program. Same program on all cores (SPMD)."""
    import concourse.bass as bass
    import concourse.tile as tile
    from concourse import bacc, masks, mybir
    from concourse.alu_op_type import AluOpType as ALU

    F32 = mybir.dt.float32
    I32 = mybir.dt.int32
    BF16 = mybir.dt.bfloat16
    AX = mybir.AxisListType

    HB = 128                 # hadamard block
    NB = din // HB           # k-blocks (32)
    NM = rows // 128         # m-tiles (4)
    NC = din // 512          # quant chunks (4 blocks each) (8)
    NG = dout // 512         # output groups (8)
    NT = 4                   # 128-wide n-tiles per group
    BITS4 = int(np.float32(4.0).view(np.int32))

    nc = bacc.Bacc("TRN2", target_bir_lowering=False, debug=False)
    xT_d = nc.dram_tensor("xT", (din, rows), F32, kind="ExternalInput")
    ss_d = nc.dram_tensor("smooth", (din,), F32, kind="ExternalInput")
    si_d = nc.dram_tensor("sin", (din,), F32, kind="ExternalInput")
    h_d = nc.dram_tensor("hb", (HB, HB), F32, kind="ExternalInput")
    wT_d = nc.dram_tensor("wT", (din, dout), F32, kind="ExternalInput")
    b_d = nc.dram_tensor("bias", (dout,), F32, kind="ExternalInput")
    oT_d = nc.dram_tensor("outT", (dout, rows), F32, kind="ExternalOutput")

    with tile.TileContext(nc) as tc, ExitStack() as ctx:
        cpool = ctx.enter_context(tc.tile_pool(name="const", bufs=1))
        ident_f = cpool.tile([128, 128], F32)
        masks.make_identity(nc, ident_f[:])
        ident_b = cpool.tile([128, 128], BF16)
        masks.make_identity(nc, ident_b[:])

        # s = smooth * S_in laid out [128 (k within block), NB (block)]
        sst = cpool.tile([NB, HB], F32)
        sit = cpool.tile([NB, HB], F32)
        nc.sync.dma_start(sst[:], ss_d[:].rearrange("(b p) -> b p", p=HB))
        nc.sync.dma_start(sit[:], si_d[:].rearrange("(b p) -> b p", p=HB))
        svn = cpool.tile([NB, HB], F32)
        nc.vector.tensor_tensor(svn[:], sst[:], sit[:], op=ALU.mult)
        sv = cpool.tile([128, NB], F32)
        # bias laid out [128 (n within tile), NG*NT (tile)]
        bnat = cpool.tile([NG * NT, HB], F32)
        nc.sync.dma_start(bnat[:], b_d[:].rearrange("(t p) -> t p", p=HB))
        biasT = cpool.tile([128, NG * NT], F32)
        with tc.tile_pool(name="sv_ps", bufs=1, space="PSUM") as svps_pool:
            svp = svps_pool.tile([128, NB], F32)
            nc.tensor.transpose(svp[:], svn[:], ident_f[:NB, :NB])
            nc.scalar.copy(sv[:], svp[:])
            btp = svps_pool.tile([128, NG * NT], F32)
            nc.tensor.transpose(btp[:], bnat[:], ident_f[: NG * NT, : NG * NT])
            nc.scalar.copy(biasT[:], btp[:])

        # H'[k, b*128+l] = H[k, l] * s[k, b]  (per-partition scalar multiply,
        # on ACT so DVE can start quantizing immediately)
        ht = cpool.tile([128, HB], F32)
        nc.sync.dma_start(ht[:], h_d[:, :])
        hp = cpool.tile([128, NB * HB], F32)
        for b in range(NB):
            nc.scalar.mul(hp[:, b * HB : (b + 1) * HB], ht[:], sv[:, b : b + 1])

        # per-block quantized-transposed activations: xqt[b] is [k, m] bf16
        xqt = [
            cpool.tile([128, rows], BF16, name=f"xqt{b}", tag=f"xqt{b}")
            for b in range(NB)
        ]

        def _emit_body():
            with (
                tc.tile_pool(name="wload", bufs=2) as wpool,
                tc.tile_pool(name="xload", bufs=2) as xpool,
                tc.tile_pool(name="rot_ps", bufs=2, space="PSUM") as rot_pool,
                tc.tile_pool(name="xqt_ps", bufs=2, space="PSUM") as xqtps_pool,
                tc.tile_pool(name="acc", bufs=1, space="PSUM") as acc_pool,
                tc.tile_pool(name="q", bufs=3) as qpool,
                tc.tile_pool(name="qs", bufs=3) as qspool,
                tc.tile_pool(name="ot", bufs=4) as opool,
            ):
                def w_load(g):
                    # one full-group load: 2KB descriptors (best SDMA rate)
                    wt = wpool.tile([128, NB * 512], BF16, name="wt", tag="wt")
                    nc.gpsimd.dma_start(
                        wt[:].rearrange("p (b n) -> p b n", n=512),
                        wT_d[:, g * 512 : (g + 1) * 512].rearrange(
                            "(b p) n -> p b n", p=128
                        ),
                    )
                    return wt

                def mm_group(wt, b, accs):
                    for nt in range(NT):
                        nc.tensor.matmul(
                            accs[nt][:],
                            wt[:, b * 512 + nt * 128 : b * 512 + (nt + 1) * 128],
                            xqt[b][:],
                            start=(b == 0),
                            stop=(b == NB - 1),
                        )

                def out_group(g, accs):
                    for nt in range(NT):
                        ot = opool.tile([128, rows], F32, name="ot", tag="ot")
                        nc.scalar.activation(
                            ot[:],
                            accs[nt][:],
                            mybir.ActivationFunctionType.Identity,
                            bias=biasT[:, g * NT + nt : g * NT + nt + 1],
                            scale=1.0,
                        )
                        nc.sync.dma_start(
                            oT_d[g * 512 + nt * 128 : g * 512 + (nt + 1) * 128, :],
                            ot[:],
                        )

                # ---- prep interleaved with output-group 0 accumulation ----
                wt0 = w_load(0)
                accs = [
                    acc_pool.tile([128, rows], F32, name=f"acc{t}", tag=f"acc{t}")
                    for t in range(NT)
                ]
                for c in range(NC):
                    xts = []
                    for jj in range(4):
                        b = 4 * c + jj
                        xt = xpool.tile([128, rows], F32, name=f"xt{jj}", tag=f"xt{jj}")
                        nc.sync.dma_start(xt[:], xT_d[b * 128 : (b + 1) * 128, :])
                        xts.append(xt)
                    xqns = []
                    for mt in range(NM):
                        rps = rot_pool.tile([128, 512], F32, name="rps", tag="rps")
                        for jj in range(4):
                            b = 4 * c + jj
                            nc.tensor.matmul(
                                rps[:, jj * 128 : (jj + 1) * 128],
                                xts[jj][:, mt * 128 : (mt + 1) * 128],
                                hp[:, b * HB : (b + 1) * HB],
                                start=True,
                                stop=True,
                            )
                        # ---- NVFP4 quant of rps [128, 512] (16-groups on free)
                        r3 = rps[:].rearrange("p (g s) -> p g s", s=16)
                        amax = qspool.tile([128, 32], F32, name="amax", tag="amax")
                        nc.vector.tensor_reduce(
                            amax[:], r3, axis=AX.X, op=ALU.max,
                            apply_absolute_value=True,
                        )
                        s12 = qspool.tile([128, 32], F32, name="s12", tag="s12")
                        nc.vector.tensor_scalar(
                            s12[:], amax[:], 1.0 / 12.0, 1e-12 / 12.0,
                            op0=ALU.mult, op1=ALU.max,
                        )
                        inv12 = qspool.tile([128, 32], F32, name="inv12", tag="inv12")
                        nc.vector.reciprocal(inv12[:], s12[:])
                        inv_bc = inv12[:].unsqueeze(2).broadcast_to((128, 32, 16))
                        s12_bc = s12[:].unsqueeze(2).broadcast_to((128, 32, 16))

                        ys2 = qpool.tile([128, 512], F32, name="ys2", tag="ys2")
                        nc.vector.tensor_tensor(
                            ys2[:].rearrange("p (g s) -> p g s", s=16), r3, inv_bc,
                            op=ALU.mult,
                        )
                        ra = qpool.tile([128, 512], F32, name="ra", tag="ra")
                        nc.scalar.activation(
                            ra[:], ys2[:], mybir.ActivationFunctionType.Abs
                        )
                        sgn = qpool.tile([128, 512], F32, name="sgn", tag="sgn")
                        nc.scalar.sign(sgn[:], ys2[:])
                        # A-branch: e = min(y2, 4.5) + M
                        e = qpool.tile([128, 512], F32, name="e", tag="e")
                        nc.vector.tensor_scalar(
                            e[:], ra[:], 4.5, F32_M_MAGIC, op0=ALU.min, op1=ALU.add
                        )
                        # B-branch (int): bitround(max(y2, 4.0))
                        g1 = qpool.tile([128, 512], I32, name="g1", tag="g1")
                        nc.vector.tensor_scalar(
                            g1[:], ra[:].bitcast(I32), BITS4, 0x00200000,
                            op0=ALU.max, op1=ALU.add,
                        )
                        b2 = qpool.tile([128, 512], F32, name="b2", tag="b2")
                        nc.vector.tensor_scalar(
                            b2[:].bitcast(I32), g1[:], -0x400000, None,
                            op0=ALU.bitwise_and,
                        )
                        # r2m4 = (e - (M+4)) + b2 == 2*level
                        r2m4 = qpool.tile([128, 512], F32, name="r2m4", tag="r2m4")
                        nc.vector.scalar_tensor_tensor(
                            r2m4[:], e[:], F32_M_MAGIC + 4.0, b2[:],
                            op0=ALU.subtract, op1=ALU.add,
                        )
                        # xq = 2*level * (amax/12) * sign
                        xqm = qpool.tile([128, 512], F32, name="xqm", tag="xqm")
                        nc.vector.tensor_tensor(
                            xqm[:].rearrange("p (g s) -> p g s", s=16),
                            r2m4[:].rearrange("p (g s) -> p g s", s=16),
                            s12_bc, op=ALU.mult,
                        )
                        xqn = qpool.tile(
                            [128, 512], BF16, name=f"xqn{mt}", tag=f"xqn{mt}"
                        )
                        nc.gpsimd.tensor_tensor(xqn[:], xqm[:], sgn[:], op=ALU.mult)
                        xqns.append(xqn)
                    # transpose quantized chunk into per-block xqt tiles
                    for jj in range(4):
                        b = 4 * c + jj
                        qps = xqtps_pool.tile([128, 512], BF16, name="qps", tag="qps")
                        for mt in range(NM):
                            nc.tensor.transpose(
                                qps[:, mt * 128 : (mt + 1) * 128],
                                xqns[mt][:, jj * 128 : (jj + 1) * 128],
                                ident_b[:],
                            )
                        nc.scalar.copy(xqt[b][:], qps[:])
                    # group-0 main accumulation for this chunk's blocks
                    for jj in range(4):
                        mm_group(wt0, 4 * c + jj, accs)
                    if c == 1:
                        wt_next = w_load(1)
                out_group(0, accs)

                # ---- remaining output groups -----------------------------
                for g in range(1, NG):
                    wt = wt_next
                    if g < NG - 1:
                        wt_next = w_load(g + 1)
                    accs = [
                        acc_pool.tile([128, rows], F32, name=f"acc{t}", tag=f"acc{t}")
                        for t in range(NT)
                    ]
                    for b in range(NB):
                        mm_group(wt, b, accs)
                    out_group(g, accs)

        for _rep in range(repeat):
            _emit_body()

    nc.compile()
    return nc


def _get_program(rows, din, dout):
    key = (rows, din, dout)
    if key not in _CACHE:
        _CACHE[key] = _build(rows, din, dout)
    return _CACHE[key]


def kernel(x, smooth_scale, S_in, H_block, w_quantized, bias):
    from concourse import bass_utils

    B, S, DIN = x.shape
    DOUT = w_quantized.shape[0]
    n_cores = 8
    rows_total = B * S
    rows = rows_total // n_cores

    nc = _get_program(rows, DIN, DOUT)

    xf = np.asarray(x, dtype=np.float32).reshape(rows_total, DIN)
    ss = np.ascontiguousarray(smooth_scale, dtype=np.float32)
    si = np.ascontiguousarray(S_in, dtype=np.float32)
    hb = np.ascontiguousarray(H_block, dtype=np.float32)
    wT = np.ascontiguousarray(np.asarray(w_quantized, dtype=np.float32).T)
    bs = np.ascontiguousarray(bias, dtype=np.float32)

    in_maps = []
    for i in range(n_cores):
        in_maps.append(
            {
                "xT": np.ascontiguousarray(xf[i * rows : (i + 1) * rows].T),
                "smooth": ss,
                "sin": si,
                "hb": hb,
                "wT": wT,
                "bias": bs,
            }
        )
    res = bass_utils.run_bass_kernel_spmd(nc, in_maps, core_ids=list(range(n_cores)))
    out = np.concatenate([r["outT"].T for r in res.results], axis=0)
    return np.ascontiguousarray(out.reshape(B, S, DOUT).astype(np.float32))

